# revision 59
# baseline (speedup 1.0000x reference)
"""Multi-head attention (embed 1024, 16 heads x 64) on 8 TRN2 NeuronCores.

Sharding: tensor-parallel over heads — each core owns 2 heads end-to-end
(qkv projection columns + attention), then per-(batch, head) AllToAlls
redistribute the per-head attention outputs so each core computes the
out-projection for its 256-token slice of each batch.

Compute is bf16 on the TensorEngine (fp32 PSUM accumulation). The engines
execute statically-ordered instruction streams, so emission order is
software-pipelined: projection chunks and out-projection slices are woven
between attention iterations, and each iteration's softmax normalization
(reciprocal + broadcast + multiply) is deferred by one slot so the slow
one-partition reciprocal never blocks the PE or DVE streams.

Layout:
  - host passes x TRANSPOSED [e, t] so projections contract e on partitions
    with no on-device transpose; projections produce Q/K/V as [head_dim, t].
  - scores are computed transposed: St[tk, tq] = Kt.T @ Qt, so the softmax
    sum over tk comes free from a ones-column appended to V: the PV matmul
    outputs [65, tq] with row 64 = sum of exp.
  - normalization: numerator+denominator copied to SBUF (frees the PV PSUM
    accumulator), then reciprocal + K=1 broadcast matmul + vector multiply.
  - out_proj consumes the AllToAll output directly (head-dim on partitions)
    and the result is PE-transposed back to row-major before the output DMA.
"""

import numpy as np
import ml_dtypes

import concourse.bass as bass
import concourse.tile as tile
from concourse import bacc, mybir
from concourse.bass_utils import run_bass_kernel_spmd
from concourse.masks import make_identity


N_CORES = 8
B, S, D = 2, 2048, 1024
T = B * S              # 4096 flattened tokens
HEADS = 16
DH = 64                # head dim
HPC = HEADS // N_CORES  # heads per core = 2
CW = HPC * DH          # per-core qkv width = 128
SCALE = DH ** -0.5
TC = T // N_CORES      # per-core output rows = 512 (256 per batch)
NW = TC // 2           # tokens per batch slice = 256
ET = D // 128          # e partition tiles = 8
F32 = mybir.dt.float32
BF16 = mybir.dt.bfloat16
EXP = mybir.ActivationFunctionType.Exp
BF = ml_dtypes.bfloat16

_CACHED_NC = None


def build():
    nc = bacc.Bacc(
        "TRN2",
        target_bir_lowering=False,
        debug=False,
        num_devices=N_CORES,
    )
    xt_ap = nc.dram_tensor("xt", [D, T], BF16, kind="ExternalInput").ap()
    wq_ap = nc.dram_tensor("wq", [D, CW], BF16, kind="ExternalInput").ap()
    wk_ap = nc.dram_tensor("wk", [D, CW], BF16, kind="ExternalInput").ap()
    wv_ap = nc.dram_tensor("wv", [D, CW], BF16, kind="ExternalInput").ap()
    bq_ap = nc.dram_tensor("bq", [CW, 1], F32, kind="ExternalInput").ap()
    bk_ap = nc.dram_tensor("bk", [CW, 1], F32, kind="ExternalInput").ap()
    bv_ap = nc.dram_tensor("bv", [CW, 1], F32, kind="ExternalInput").ap()
    wout_ap = nc.dram_tensor("wout", [D, D], BF16, kind="ExternalInput").ap()
    bout_ap = nc.dram_tensor("bout", [128, ET], F32, kind="ExternalInput").ap()
    out_ap = nc.dram_tensor("out", [TC, D], F32, kind="ExternalOutput").ap()

    with tile.TileContext(nc) as tc:
        with (
            tc.tile_pool(name="singles", bufs=1) as singles,
            tc.tile_pool(name="xt", bufs=64) as xt_pool,
            tc.tile_pool(name="vt", bufs=2) as vt_pool,
            tc.tile_pool(name="exp", bufs=6) as exp_pool,
            tc.tile_pool(name="fo", bufs=2) as fo_pool,
            tc.tile_pool(name="small", bufs=2) as small_pool,
            tc.tile_pool(name="mmps", bufs=2, space="PSUM") as mmps,
            tc.tile_pool(name="stps", bufs=2, space="PSUM") as stps,
            tc.tile_pool(name="pvps", bufs=1, space="PSUM") as pvps,
            tc.tile_pool(name="dram", bufs=1, space="DRAM") as dram,
        ):
            # A2A bounce buffers, one pair per (batch, head): shard j holds
            # tokens [j*256,(j+1)*256) of batch b, 64 head-dims per shard.
            dummy_in = dram.tile([N_CORES, 16], BF16, name="dummy_in")
            dummy_out = dram.tile([N_CORES, 16], BF16, name="dummy_out")
            a2a_in = [
                [dram.tile([N_CORES * DH, NW], BF16, name=f"a2a_in{b}_{h}")
                 for h in range(HPC)] for b in range(2)
            ]
            a2a_out = [
                [dram.tile([N_CORES * DH, NW], BF16, name=f"a2a_out{b}_{h}")
                 for h in range(HPC)] for b in range(2)
            ]

            # ---- constants / weights resident in SBUF ----
            identb = singles.tile([128, 128], BF16)
            make_identity(nc, identb)
            ident32 = singles.tile([128, 128], F32)
            make_identity(nc, ident32)
            ones64 = singles.tile([1, DH], BF16)
            nc.vector.memset(ones64, 1.0)

            w_sb, b_sb = {}, {}
            for name, wap, bap in (
                ("q", wq_ap, bq_ap), ("k", wk_ap, bk_ap), ("v", wv_ap, bv_ap)
            ):
                w_sb[name] = singles.tile(
                    [128, ET, CW], BF16, tag=f"w{name}", name=f"w{name}_sb"
                )
                nc.gpsimd.dma_start(
                    out=w_sb[name],
                    in_=wap.rearrange("(et p) c -> p et c", p=128),
                )
                b_sb[name] = singles.tile(
                    [CW, 1], F32, tag=f"b{name}", name=f"b{name}_sb"
                )
                nc.gpsimd.dma_start(out=b_sb[name], in_=bap)
            bout_sb = singles.tile([128, ET], F32)
            nc.gpsimd.dma_start(out=bout_sb, in_=bout_ap)
            wout_sb = singles.tile([128, ET, D], BF16, tag="wout")

            # persistent activations
            qt = singles.tile([CW, T], BF16, tag="qt")   # [2h*64, t] transposed Q
            kt = singles.tile([CW, T], BF16, tag="kt")
            # partition-swapped copy of kt: alternating score matmuls load
            # weights into the OTHER PE-array row group, so each LDWEIGHTS
            # pulls ahead of the in-flight matmul instead of serializing
            kt2 = singles.tile([CW, T], BF16, tag="kt2")
            qt2 = singles.tile([CW, T], BF16, tag="qt2")
            # V natural per head, 65-wide tk-tiles (col 64 = ones for denom)
            vsb = [
                singles.tile(
                    [128, T // 128, DH + 1], BF16, tag=f"v{h}", name=f"v{h}_sb"
                )
                for h in range(HPC)
            ]
            for h in range(HPC):
                nc.vector.memset(vsb[h][:, :, DH:DH + 1], 1.0)
            # gathered head-features for this core's token rows, per batch
            g_sb = [
                singles.tile([128, ET, NW], BF16, tag=f"g{b}", name=f"g{b}_sb")
                for b in range(2)
            ]

            xt_view = xt_ap.rearrange("(et p) t -> p et t", p=128)

            chunk_xt = {}

            def emit_chunk_dma(tch):
                xt_e = []
                for et in range(ET):
                    xe = xt_pool.tile([128, 512], BF16, tag="xt", name="xt_e")
                    nc.sync.dma_start(
                        out=xe,
                        in_=xt_view[:, et, tch * 512:(tch + 1) * 512],
                    )
                    xt_e.append(xe)
                chunk_xt[tch] = xt_e

            def emit_proj(tch, name):
                """One projection (q/k/v) of a 512-token chunk — emitted as a
                separate piece so chunks weave between attention tk-groups
                without starving the exp pipeline."""
                xt_e = chunk_xt[tch]
                dest = {"q": qt, "k": kt, "v": None}[name]
                pp = mmps.tile([CW, 512], F32, tag="mm", name="pp_proj")
                for et in range(ET):
                    nc.tensor.matmul(
                        pp,
                        w_sb[name][:, et, :],
                        xt_e[et],
                        start=(et == 0),
                        stop=(et == ET - 1),
                    )
                if dest is not None:
                    nc.vector.tensor_scalar_add(
                        dest[:, tch * 512:(tch + 1) * 512], pp, b_sb[name]
                    )
                    src_t, dst_t = (kt, kt2) if name == "k" else (qt, qt2)
                    sl = slice(tch * 512, (tch + 1) * 512)
                    nc.gpsimd.dma_start(
                        out=dst_t[DH:2 * DH, sl], in_=src_t[0:DH, sl]
                    )
                    nc.gpsimd.dma_start(
                        out=dst_t[0:DH, sl], in_=src_t[DH:2 * DH, sl]
                    )
                else:
                    vt_tmp = vt_pool.tile([CW, 512], BF16, name="vt_tmp")
                    nc.vector.tensor_scalar_add(vt_tmp, pp, b_sb[name])
                    for tt in range(4):
                        ps2 = mmps.tile([128, 128], BF16, tag="mm", name="ps_vtr")
                        nc.tensor.transpose(
                            ps2, vt_tmp[:, tt * 128:(tt + 1) * 128], identb
                        )
                        ttg = tch * 4 + tt
                        for h in range(HPC):
                            nc.vector.tensor_copy(
                                vsb[h][:, ttg, 0:DH],
                                ps2[:, h * DH:(h + 1) * DH],
                            )

            def emit_chunk(tch):
                emit_chunk_dma(tch)
                for name in ("q", "k", "v"):
                    emit_proj(tch, name)

            def weave(b, h, tqh, fillers):
                """Attention iteration with filler pieces spread between
                tk-tile groups so ScalarE's exp stream never starves while
                the PE works through a filler."""
                A = AttIter(b, h, tqh)
                k = len(fillers)
                for i, f in enumerate(fillers):
                    A.advance((i + 1) * 16 // (k + 1))
                    f()
                return A.finish()

            class AttIter:
                """Resumable attention iteration: 1024 queries of head h,
                batch b. advance(tk_hi) emits score/exp/PV work for tk-tiles
                up to tk_hi (PV pipelined one tile behind scores); finish()
                drains and returns (pvc, recip) for deferred normalization.
                Splitting lets the first iteration start as soon as the
                chunks covering its early tk-tiles are projected."""

                def __init__(self, b, h, tqh):
                    self.b, self.h = b, h
                    self.po = h * DH
                    self.po2 = DH - self.po
                    self.tq0 = b * S + tqh * 1024
                    self.pv = pvps.tile([DH + 1, 1024], F32, name="pv")
                    self.exs = {}
                    self.sc_done = 0
                    self.pv_done = 0

                def _scores(self, tkt):
                    st = stps.tile([128, 1024], F32, tag="st", name="st")
                    k0 = self.b * S + tkt * 128
                    for nh in range(2):
                        if nh == 0:
                            lhsT = kt[self.po:self.po + DH, k0:k0 + 128]
                            rhs_q = qt[self.po:self.po + DH,
                                       self.tq0 + nh * 512:
                                       self.tq0 + (nh + 1) * 512]
                        else:
                            lhsT = kt2[self.po2:self.po2 + DH, k0:k0 + 128]
                            rhs_q = qt2[self.po2:self.po2 + DH,
                                        self.tq0 + nh * 512:
                                        self.tq0 + (nh + 1) * 512]
                        nc.tensor.matmul(
                            st[:, nh * 512:(nh + 1) * 512], lhsT, rhs_q
                        )
                    ex = exp_pool.tile([128, 1024], BF16, name="ex")
                    nc.scalar.activation(ex, st, EXP)
                    self.exs[tkt] = ex

                def _pvacc(self, tkt):
                    ex = self.exs.pop(tkt)
                    for nh in range(2):
                        nc.tensor.matmul(
                            self.pv[:, nh * 512:(nh + 1) * 512],
                            vsb[self.h][:, self.b * 16 + tkt, :],
                            ex[:, nh * 512:(nh + 1) * 512],
                            start=(tkt == 0),
                            stop=(tkt == 15),
                        )

                def advance(self, tk_hi):
                    while self.sc_done < tk_hi:
                        self._scores(self.sc_done)
                        self.sc_done += 1
                        if self.pv_done < self.sc_done - 1:
                            self._pvacc(self.pv_done)
                            self.pv_done += 1

                def finish(self):
                    self.advance(16)
                    while self.pv_done < 16:
                        self._pvacc(self.pv_done)
                        self.pv_done += 1
                    # copy numerator + denominator off PSUM so pv frees now;
                    # the denominator is DMA-spread across 128 partitions so
                    # the reciprocal runs 128-wide (a 1-lane [1,1024]
                    # reciprocal costs 6.5us on the DVE)
                    pvc = fo_pool.tile([DH, 1024], F32, tag="pvc", name="pvc")
                    nc.vector.tensor_copy(pvc, self.pv[0:DH, :])
                    dn = small_pool.tile([1, 1024], F32, tag="dn", name="dn")
                    nc.vector.tensor_copy(dn, self.pv[DH:DH + 1, :])
                    dn128 = small_pool.tile(
                        [128, 8], F32, tag="dn128", name="dn128"
                    )
                    nc.gpsimd.dma_start(out=dn128, in_=dn)
                    r128 = small_pool.tile([128, 8], BF16, tag="r128", name="r128")
                    with nc.allow_low_precision(
                        reason="softmax denom reciprocal feeds bf16 bcast"
                    ):
                        nc.vector.reciprocal(r128, dn128)
                    recip = small_pool.tile([1, 1024], BF16, name="recip")
                    nc.gpsimd.dma_start(out=recip, in_=r128)
                    return pvc, recip

            def emit_attention(b, h, tqh):
                return AttIter(b, h, tqh).finish()

            def emit_finish(b, h, tqh, pvc, recip):
                """Deferred normalization + scatter into the A2A input."""
                po = h * DH
                fo = fo_pool.tile([DH, 1024], BF16, name="fo")
                for nh in range(2):
                    bc = mmps.tile([DH, 512], F32, tag="mm", name="bc")
                    nc.tensor.matmul(
                        bc, ones64, recip[:, nh * 512:(nh + 1) * 512]
                    )
                    nc.vector.tensor_mul(
                        fo[:, nh * 512:(nh + 1) * 512],
                        pvc[:, nh * 512:(nh + 1) * 512],
                        bc,
                    )
                # head h of batch b goes to a2a_in[b][h]: shard j = rank j's
                # 256 tokens, 64 rows each
                a2a_view = a2a_in[b][h][:, :].rearrange("(j p) t -> p j t", p=DH)
                j0 = tqh * 4
                nc.sync.dma_start(
                    out=a2a_view[:, j0:j0 + 4, :],
                    in_=fo.rearrange("p (j t) -> p j t", j=4),
                )

            def emit_a2a(b, h):
                nc.gpsimd.collective_compute(
                    "AllToAll",
                    mybir.AluOpType.bypass,
                    replica_groups=[list(range(N_CORES))],
                    ins=[a2a_in[b][h][:, :].opt()],
                    outs=[a2a_out[b][h][:, :].opt()],
                )

            def emit_gather(b, h):
                # rank r's shard lands at rows r*64..(r+1)*64 → head h's dims
                # are partitions h*64..h*64+64 of g_sb[b][:, r, :]. Emitted
                # just before the first consumer: this DMA waits on the
                # collective, and the sync engine issues triggers in order —
                # an early emission would stall every later DMA behind it.
                nc.sync.dma_start(
                    out=g_sb[b][h * DH:(h + 1) * DH, :, :],
                    in_=a2a_out[b][h][:, :].rearrange("(r p) t -> p r t", p=DH),
                )

            def emit_outproj(b, ot):
                """Out-projection columns [ot*128,(ot+1)*128) for batch b."""
                pp = mmps.tile([128, NW], F32, tag="mm", name="pp_out")
                for ht in range(ET):
                    nc.tensor.matmul(
                        pp,
                        wout_sb[:, ht, ot * 128:(ot + 1) * 128],
                        g_sb[b][:, ht, :],
                        start=(ht == 0),
                        stop=(ht == ET - 1),
                    )
                ob = vt_pool.tile([128, NW], F32, tag="ob", name="ob")
                nc.vector.tensor_scalar_add(ob, pp, bout_sb[:, ot:ot + 1])
                for tt in range(2):
                    ps = mmps.tile([128, 128], F32, tag="mm", name="ps_otr")
                    nc.tensor.transpose(
                        ps, ob[:, tt * 128:(tt + 1) * 128], ident32
                    )
                    on = small_pool.tile([128, 128], F32, tag="on", name="on")
                    nc.vector.tensor_copy(on, ps)
                    nc.sync.dma_start(
                        out=out_ap[b * NW + tt * 128:b * NW + (tt + 1) * 128,
                                   ot * 128:(ot + 1) * 128],
                        in_=on,
                    )

            # ---- emission schedule: software-pipelined. fin(i) lands one
            # attention iteration after att(i) so the reciprocal is ready
            # before its broadcast matmul enters the PE stream. ----
            # tiny dummy collective up front: absorbs the collective-stream
            # entry barrier + first-trigger latency while the PE is still
            # ramping, so the first real AllToAll fires instantly
            nc.gpsimd.collective_compute(
                "AllToAll",
                mybir.AluOpType.bypass,
                replica_groups=[list(range(N_CORES))],
                ins=[dummy_in[:, :].opt()],
                outs=[dummy_out[:, :].opt()],
            )
            ATT0 = [(0, h, tqh) for h in range(HPC) for tqh in range(2)]
            ATT1 = [(1, h, tqh) for h in range(HPC) for tqh in range(2)]
            pend = {}

            # batch 0: the first attention iteration starts as soon as the
            # chunks covering its tk-tiles land; later iterations weave
            # chunk/finish/out-proj pieces between tk-groups so neither the
            # PE nor ScalarE sits idle at iteration boundaries
            emit_chunk(0)
            emit_chunk(1)
            A0 = AttIter(*ATT0[0])
            A0.advance(8)
            emit_chunk(2)
            A0.advance(12)
            emit_chunk(3)
            pend[0] = A0.finish()
            emit_chunk(4)
            pend[1] = emit_attention(*ATT0[1])
            emit_finish(*ATT0[0], *pend[0])
            emit_chunk(5)
            pend[2] = emit_attention(*ATT0[2])
            emit_finish(*ATT0[1], *pend[1])
            emit_a2a(0, 0)
            nc.gpsimd.dma_start(
                out=wout_sb, in_=wout_ap.rearrange("(ht p) o -> p ht o", p=128)
            )
            emit_chunk(6)
            pend[3] = emit_attention(*ATT0[3])
            emit_finish(*ATT0[2], *pend[2])
            emit_chunk(7)
            # batch 1 attention, interleaved with batch-0 out-projection
            pend[4] = emit_attention(*ATT1[0])
            emit_finish(*ATT0[3], *pend[3])
            emit_a2a(0, 1)
            pend[5] = emit_attention(*ATT1[1])
            emit_finish(*ATT1[0], *pend[4])
            emit_gather(0, 0)
            emit_gather(0, 1)
            emit_outproj(0, 0)
            emit_outproj(0, 1)
            pend[6] = emit_attention(*ATT1[2])
            emit_finish(*ATT1[1], *pend[5])
            emit_a2a(1, 0)
            emit_outproj(0, 2)
            emit_outproj(0, 3)
            pend[7] = emit_attention(*ATT1[3])
            # fire the last A2A as early as possible — everything after it
            # is the exposed tail
            emit_finish(*ATT1[2], *pend[6])
            emit_finish(*ATT1[3], *pend[7])
            emit_a2a(1, 1)
            emit_gather(1, 0)
            for ot in range(4, 8):
                emit_outproj(0, ot)
            emit_gather(1, 1)
            for ot in range(ET):
                emit_outproj(1, ot)
    nc.compile()
    return nc


def shard_inputs(x, w_qkv, b_qkv, w_out, b_out):
    """Split full inputs into the 8 per-core input maps (bf16 compute).
    x is transposed host-side so projections need no on-device transpose."""
    x2d = np.asarray(x, dtype=np.float32).reshape(T, D)
    xt = np.ascontiguousarray(x2d.T.astype(BF))  # [D, T]
    w_qkv = np.asarray(w_qkv, dtype=np.float32)
    b_qkv = np.asarray(b_qkv, dtype=np.float32)
    w_out = np.ascontiguousarray(np.asarray(w_out, dtype=np.float32).astype(BF))
    b_out = np.asarray(b_out, dtype=np.float32)
    bout_r = np.ascontiguousarray(b_out.reshape(ET, 128).T)  # [p, ot]
    in_maps = []
    for i in range(N_CORES):
        c0 = i * CW
        wq = np.ascontiguousarray(w_qkv[:, c0:c0 + CW] * SCALE).astype(BF)
        wk = np.ascontiguousarray(w_qkv[:, D + c0:D + c0 + CW]).astype(BF)
        wv = np.ascontiguousarray(w_qkv[:, 2 * D + c0:2 * D + c0 + CW]).astype(BF)
        bq = (b_qkv[c0:c0 + CW] * SCALE).reshape(CW, 1)
        bk = b_qkv[D + c0:D + c0 + CW].reshape(CW, 1)
        bv = b_qkv[2 * D + c0:2 * D + c0 + CW].reshape(CW, 1)
        in_maps.append({
            "xt": xt,
            "wq": wq, "wk": wk, "wv": wv,
            "bq": np.ascontiguousarray(bq),
            "bk": np.ascontiguousarray(bk),
            "bv": np.ascontiguousarray(bv),
            "wout": w_out,
            "bout": bout_r,
        })
    return in_maps


def get_nc():
    global _CACHED_NC
    if _CACHED_NC is None:
        _CACHED_NC = build()
    return _CACHED_NC


def run(in_maps, trace=False, **kw):
    nc = get_nc()
    return run_bass_kernel_spmd(
        nc, in_maps, core_ids=list(range(N_CORES)), trace=trace, **kw
    )


def assemble(results):
    """Each core returns [512, 1024]: rows 0..255 = its 256-token slice of
    batch 0, rows 256..511 = its slice of batch 1."""
    out = np.empty((T, D), dtype=np.float32)
    for i, r in enumerate(results):
        o = r["out"]
        out[i * NW:(i + 1) * NW] = o[:NW]
        out[S + i * NW:S + (i + 1) * NW] = o[NW:]
    return out.reshape(B, S, D)


def kernel(x, w_qkv, b_qkv, w_out, b_out):
    in_maps = shard_inputs(x, w_qkv, b_qkv, w_out, b_out)
    res = run(in_maps, trace=False)
    return assemble(res.results)
